# revision 20
# baseline (speedup 1.0000x reference)
"""Trainium2 Bass kernel for a masked (reset-gated) LSTM.

Reference computation (PyTorch-gate-order LSTM with per-step reset mask):
    mask = 1 - reset                    # (S, B, 1)
    per step t: h *= m_t; c *= m_t
                gates = x_t @ W_ih.T + h @ W_hh.T + b_ih + b_hh
                i, f, g, o = split(gates, 4)
                c = sig(f)*c + sig(i)*tanh(g)
                h = sig(o)*tanh(c)
    out = stack(h_t) + x ; returns (out, hT, cT)

Shapes: S=256, B=128, DIM=UNITS=1024.

Distribution across the 8 NeuronCores: 4 batch groups (32 each) x 2
sequence halves (128 steps each).  A reset (mask==0) cuts the recurrence
exactly, so the second-half cores start W "warmup" steps early from zero
state; W is chosen on the host from the actual reset data so that every
(batch, boundary) pair has a reset inside the warmup window, making the
split exact, not approximate.

Per core, each step runs the fused matmul
    gates[b, :] = [x_t; 1; h] @ [W_ih.T; bias; W_hh.T]   (contraction 17x128)
as 4 column-tiled concurrent matmul groups (M=32 batch each), yielding a
PSUM layout [128 = 4 groups x 32 batch, 1024 = 2 halves x {i,f,o,g} x 128]
where all four gates of a (batch, unit) pair live in the same partition.
Elementwise LSTM cell math then runs at full 128-lane utilization, and the
next step's stationary h.T chunks are produced by 4 row-packed PE
transposes per half.
"""

import os
import numpy as np
import ml_dtypes

import concourse.bass as bass
import concourse.tile as tile
from concourse import bacc, mybir
from concourse.bass_utils import run_bass_kernel_spmd
from concourse.masks import make_identity
from concourse.tile_rust import add_dep_helper

BF16 = mybir.dt.bfloat16
F32 = mybir.dt.float32
AF = mybir.ActivationFunctionType
ALU = mybir.AluOpType

S, B, DIM, UNITS = 256, 128, 1024, 1024
G4 = 4 * UNITS
NCORES = 8
NGRP = 4          # batch groups
BL = B // NGRP    # 32 batch per core
T_OUT = S // 2    # 128 real steps per core

# quarter order within a group's 512-col half-bank: i, f, o, g
_GATE_OF_QUARTER = np.array([0, 1, 3, 2])  # torch rows: i=0, f=1, g=2, o=3

LAST_RESULT = None  # BassKernelResults of the most recent run (for test harness)


def _col_w_rows():
    """For wT column c (0..4095): the W_ih/W_hh row it reads, and layout maps."""
    col = np.arange(G4)
    j = col // 1024          # col group (psum partition block)
    r = col % 1024
    hf = r // 512            # half (psum bank)
    rr = r % 512
    gi = rr // 128           # quarter index {i,f,o,g}
    ul = rr % 128
    u = 256 * j + 128 * hf + ul          # global unit
    gate = _GATE_OF_QUARTER[gi]
    return 1024 * gate + u


def _build_program(W):
    """Build the (uniform SPMD) Bass program for warmup length W."""
    T_loc = W + T_OUT
    KCH = 17  # 8 x-chunks + 1 bias chunk + 8 h-chunks

    nc = bacc.Bacc(None, target_bir_lowering=False, debug=False)
    wt_d = nc.dram_tensor("wt", [128, KCH * G4], BF16, kind="ExternalInput")
    xt_d = nc.dram_tensor("xt", [T_loc, 128, 256], BF16, kind="ExternalInput")
    xn_d = nc.dram_tensor("xn", [T_OUT, 128, 256], F32, kind="ExternalInput")
    mk_d = nc.dram_tensor("mk", [128, T_loc], F32, kind="ExternalInput")
    out_d = nc.dram_tensor("out", [T_OUT, 128, 256], F32, kind="ExternalOutput")
    ht_d = nc.dram_tensor("ht", [128, 256], F32, kind="ExternalOutput")
    ct_d = nc.dram_tensor("ct", [128, 256], F32, kind="ExternalOutput")

    with tile.TileContext(nc) as tc:
        with (
            tc.tile_pool(name="const", bufs=1) as cpool,
            tc.tile_pool(name="xt", bufs=3) as xt_pool,
            tc.tile_pool(name="xn", bufs=3) as xn_pool,
            tc.tile_pool(name="ew", bufs=2) as ew_pool,
            tc.tile_pool(name="outp", bufs=3) as out_pool,
            tc.tile_pool(name="ps", bufs=2, space="PSUM") as ps_pool,
            tc.tile_pool(name="tps", bufs=2, space="PSUM") as tps_pool,
        ):
            # -- static tiles -------------------------------------------------
            wt_sb = cpool.tile([128, KCH * G4], BF16, tag="wt")
            nc.sync.dma_start(wt_sb[:], wt_d[:])
            mk_sb = cpool.tile([128, T_loc], F32, tag="mk")
            nc.sync.dma_start(mk_sb[:], mk_d[:])

            ones_sb = cpool.tile([128, 32], BF16, tag="ones")
            nc.vector.memset(ones_sb[:], 0.0)
            nc.vector.memset(ones_sb[0:1, :], 1.0)

            ident = cpool.tile([128, 128], F32, tag="ident")
            make_identity(nc, ident[:])

            # persistent state (explicit ping-pong)
            hT_sb = [cpool.tile([128, 256], BF16, tag=f"hT{i}", name=f"hT{i}")
                     for i in range(2)]
            c_sb = [cpool.tile([128, 256], F32, tag=f"c{i}", name=f"c{i}")
                    for i in range(2)]
            nc.vector.memset(hT_sb[0][:], 0.0)
            nc.vector.memset(c_sb[0][:], 0.0)

            for t in range(T_loc):
                pe = t % 2
                hT_prev, hT_next = hT_sb[pe], hT_sb[1 - pe]
                c_prev, c_next = c_sb[pe], c_sb[1 - pe]
                is_last = t == T_loc - 1

                xt_t = xt_pool.tile([128, 256], BF16, tag="xt")
                nc.gpsimd.dma_start(xt_t[:], xt_d[t])

                ps = ps_pool.tile([128, 1024], F32, tag="ps")
                # x-chunks first so next step's matmuls can start while the
                # previous step's elementwise tail is still producing h.T.
                # Each (j, hf) accumulation group has its own start/stop
                # (per-partition zero-region semantics); the explicit chain
                # keeps the PE in emission order so every group's start=True
                # matmul executes before its accumulating matmuls.
                prev_mm = None
                for k in range(KCH):
                    if k < 8:
                        lhsT = xt_t[:, 32 * k:32 * k + 32]
                    elif k == 8:
                        lhsT = ones_sb[:, 0:32]
                    else:
                        # chunk kk (units [128*kk, 128*kk+128)) lives in the
                        # half-(kk%2) transpose output at column block kk//2
                        kk = k - 9
                        off = 128 * (kk % 2) + 32 * (kk // 2)
                        lhsT = hT_prev[:, off:off + 32]
                    for hf in range(2):
                        for j in range(4):
                            base = G4 * k + 1024 * j + 512 * hf
                            mm = nc.tensor.matmul(
                                ps[32 * j:32 * j + 32, 512 * hf:512 * hf + 512],
                                lhsT,
                                wt_sb[:, base:base + 512],
                                start=(k == 0),
                                stop=(k == KCH - 1),
                                tile_position=(0, 32 * j),
                                skip_group_check=True,
                            )
                            if prev_mm is not None:
                                add_dep_helper(mm.ins, prev_mm.ins, sync=False,
                                               reason="PE accumulation order")
                            prev_mm = mm

                A = ew_pool.tile([128, 1024], BF16, tag="A")
                h = ew_pool.tile([128, 256], F32, tag="h")
                tch = ew_pool.tile([128, 256], BF16, tag="tch")
                t1 = ew_pool.tile([128, 256], F32, tag="t1")
                t2 = ew_pool.tile([128, 256], F32, tag="t2")
                if not is_last:
                    hm = ew_pool.tile([128, 256], F32, tag="hm")
                    tps = tps_pool.tile([128, 256], F32, tag="tps")
                    tp_insts = []
                for hf in range(2):
                    o5 = 512 * hf
                    o1 = 128 * hf
                    s1 = slice(o1, o1 + 128)
                    # A = [sig(i), sig(f), sig(o), tanh(g)]
                    nc.scalar.activation(A[:, o5:o5 + 384], ps[:, o5:o5 + 384], AF.Sigmoid)
                    nc.scalar.activation(A[:, o5 + 384:o5 + 512], ps[:, o5 + 384:o5 + 512], AF.Tanh)
                    # t1 = (c * m_t) * sig(f);  t2 = sig(i) * tanh(g)
                    nc.vector.scalar_tensor_tensor(
                        t1[:, s1], c_prev[:, s1], mk_sb[:, t:t + 1],
                        A[:, o5 + 128:o5 + 256], op0=ALU.mult, op1=ALU.mult)
                    nc.vector.tensor_tensor(
                        t2[:, s1], A[:, o5:o5 + 128], A[:, o5 + 384:o5 + 512], op=ALU.mult)
                    nc.vector.tensor_add(c_next[:, s1], t1[:, s1], t2[:, s1])
                    nc.scalar.activation(tch[:, s1], c_next[:, s1], AF.Tanh)
                    # h = sig(o) * tanh(c)
                    nc.vector.tensor_tensor(
                        h[:, s1], A[:, o5 + 256:o5 + 384], tch[:, s1], op=ALU.mult)
                    if not is_last:
                        # h_m = h * m_{t+1}, then one full [128,128] PE
                        # transpose per half: the (group, batch) partition dim
                        # becomes columns, yielding all 4 h.T chunks of this
                        # half at column blocks j (chunk kk = 2j + hf)
                        nc.scalar.activation(hm[:, s1], h[:, s1], AF.Copy,
                                             scale=mk_sb[:, t + 1:t + 2])
                        tp = nc.tensor.matmul(
                            tps[:, s1],
                            hm[:, s1],
                            ident[:],
                            is_transpose=True,
                            start=(hf == 0),
                            stop=(hf == 1),
                            skip_group_check=True,
                        )
                        if tp_insts:
                            add_dep_helper(tp.ins, tp_insts[-1].ins,
                                           sync=False, reason="tps order")
                        tp_insts.append(tp)
                if not is_last:
                    nc.vector.tensor_copy(hT_next[:], tps[:])  # f32 -> bf16
                if t >= W:
                    to = t - W
                    xn_t = xn_pool.tile([128, 256], F32, tag="xn")
                    nc.gpsimd.dma_start(xn_t[:], xn_d[to])
                    out_t = out_pool.tile([128, 256], F32, tag="out")
                    nc.gpsimd.tensor_tensor(out_t[:], h[:], xn_t[:], op=ALU.add)
                    nc.gpsimd.dma_start(out_d[to], out_t[:])
                if is_last:
                    nc.sync.dma_start(ht_d[:], h[:])
                    nc.sync.dma_start(ct_d[:], c_next[:])

    if not nc.is_finalized():
        nc.finalize()
    return nc


def _prep_core_inputs(x, reset, wt_packed, W, g, s):
    """Host-side shard/layout prep for core (batch group g, seq half s)."""
    T_loc = W + T_OUT
    b0 = BL * g
    xs_all = x[:, b0:b0 + BL, :]                       # (S, 32, 1024) f32
    mask_all = 1.0 - reset[:, b0:b0 + BL].astype(np.float32)  # (S, 32)

    # xt: [T_loc, 128, 256] bf16; xt[t, p, 32c+b] = x_loc[t, b, 128c+p]
    xt = np.zeros((T_loc, 128, 256), dtype=ml_dtypes.bfloat16)
    g0 = s * T_OUT
    t0 = g0 - W  # global step of local step 0
    xs = xs_all[max(t0, 0):g0 + T_OUT]                 # real steps available
    xs_t = xs.transpose(0, 2, 1).reshape(-1, 8, 128, BL).transpose(0, 2, 1, 3)
    xt[T_loc - xs_t.shape[0]:] = xs_t.reshape(-1, 128, 256).astype(ml_dtypes.bfloat16)

    # masks: [128, T_loc]; mk[32j+b, t] = m_local[t, b] (replicated over j)
    m_loc = np.zeros((T_loc, BL), dtype=np.float32)
    m_loc[T_loc - (g0 + T_OUT - max(t0, 0)):] = mask_all[max(t0, 0):g0 + T_OUT]
    if s == 0:
        m_loc[:W + 1] = 0.0  # junk warmup + exact h0=c0=0 injection at t=0
    mk = np.tile(m_loc.T, (NGRP, 1)).astype(np.float32)          # (128, T_loc)

    # xn: [T_OUT, 128, 256] f32; xn[to, 32j+b, uu] = x_loc[g0+to, b, 256j+uu]
    xn = np.ascontiguousarray(
        xs_all[g0:g0 + T_OUT].reshape(T_OUT, BL, 4, 256).transpose(0, 2, 1, 3)
    ).reshape(T_OUT, 128, 256).astype(np.float32)

    return {"wt": wt_packed, "xt": xt, "xn": xn, "mk": mk}


def _unshuffle(a):
    """[*, 128, 256] device layout -> [*, 32, 1024] logical layout."""
    lead = a.shape[:-2]
    return np.ascontiguousarray(
        a.reshape(*lead, 4, 32, 256).swapaxes(-3, -2)
    ).reshape(*lead, 32, 1024)


def kernel(x, reset, W_ih, W_hh, b_ih, b_hh, h0, c0):
    global LAST_RESULT
    x = np.asarray(x, dtype=np.float32)
    reset = np.asarray(reset)
    W_ih = np.asarray(W_ih, dtype=np.float32)
    W_hh = np.asarray(W_hh, dtype=np.float32)
    b_ih = np.asarray(b_ih, dtype=np.float32)
    b_hh = np.asarray(b_hh, dtype=np.float32)
    assert not np.any(np.asarray(h0)) and not np.any(np.asarray(c0)), \
        "kernel assumes zero initial state (as produced by setup_inputs)"

    # warmup length: every batch column must hit a reset in [S/2 - W, S/2]
    r = reset[:T_OUT + 1, :]  # steps 0..128 inclusive
    last = np.array([np.max(np.nonzero(r[:, b])[0]) if r[:, b].any() else -1
                     for b in range(B)])
    w_need = int((T_OUT - last).max()) if (last >= 0).all() else T_OUT
    W = max(8, -(-w_need // 8) * 8)
    W = min(W, T_OUT)

    # pack combined weights + bias: Wcat [17*128, 4096] -> [128, 17*4096]
    wrow = _col_w_rows()
    KCH = 17
    wcat = np.zeros((KCH * 128, G4), dtype=np.float32)
    wcat[0:1024] = W_ih[wrow].T
    wcat[1024] = (b_ih + b_hh)[wrow]
    wcat[1152:2176] = W_hh[wrow].T
    wt_packed = np.ascontiguousarray(
        wcat.reshape(KCH, 128, G4).transpose(1, 0, 2)
    ).reshape(128, KCH * G4).astype(ml_dtypes.bfloat16)

    nc = _build_program(W)
    in_maps = []
    for core in range(NCORES):
        s, g = divmod(core, NGRP)
        in_maps.append(_prep_core_inputs(x, reset, wt_packed, W, g, s))

    res = run_bass_kernel_spmd(nc, in_maps, list(range(NCORES)))
    LAST_RESULT = res

    out = np.empty((S, B, UNITS), dtype=np.float32)
    hT = np.empty((B, UNITS), dtype=np.float32)
    cT = np.empty((B, UNITS), dtype=np.float32)
    for core in range(NCORES):
        s, g = divmod(core, NGRP)
        b0 = BL * g
        out[s * T_OUT:(s + 1) * T_OUT, b0:b0 + BL] = _unshuffle(res.results[core]["out"])
        if s == 1:
            hT[b0:b0 + BL] = _unshuffle(res.results[core]["ht"])
            cT[b0:b0 + BL] = _unshuffle(res.results[core]["ct"])
    return out, hT, cT
